# revision 14
# baseline (speedup 1.0000x reference)
"""Trainium2 Bass kernel for nn_CrossAttention (B=8, S1=S2=2048, D=512, single head).

Sharding: batch dim B=8 across the 8 NeuronCores (data parallel). Each core runs
the full cross-attention for one batch element:
    q = RoPE(h1 @ Wq.T + bq); k = RoPE(h2 @ Wk.T + bk); v = h2 @ Wv.T + bv
    out = softmax(q k^T / sqrt(D)) v @ Wo.T + bo

Design notes (v2):
  - All matmuls in bf16 (fp32 PSUM accumulation): rel_l2 vs fp32 reference ~6e-3.
  - Scores are computed TRANSPOSED (S^T[k,q]) so the probability matrix feeds the
    PV matmul directly as the moving operand - no P transposes.
  - Softmax skips max-subtraction (energies are ~N(0,1), |e| < 8, exp is safe).
  - Softmax denominators WITHOUT PE ones-matmuls: P^T tiles are accumulated
    across key blocks on DVE (even kb) and Pool (odd kb) into two fp32 SBUF
    accumulators; combined + cast to bf16, then 4 one-column matmuls
    (acc_chunk^T @ ones) drop the denominators directly onto q-partitions for
    the reciprocal. This removes 16 512-wide PE matmuls + 4 PE transposes/qt.
  - PSUM (8 banks): st 3 | ot 2 | pf 3. The "pf" tag is a 3-slot rotation of
    1-bank tiles shared by the interleaved next-q projection halves, the
    denominator column tile, and the final-projection accumulators - no
    single-buffer interlock (v1's ps_fin bufs=1 serialized PE against DVE).
  - PV is emitted with a 2-iteration skew behind the score matmuls so the ACT
    exp latency never head-of-line blocks the PE.
  - Prologue DMAs are split across all 5 engine queues with the first-needed
    chunks (wk, first h2 slice halves, first RoPE table slices) leading their
    queues: the first matmul can start as soon as ~2 chunks land instead of
    waiting for whole-tensor transfers.
  - Output is written to DRAM in bf16 (halves the tail DMA) and cast to fp32
    on host.
"""

import math
import sys

import numpy as np

for _p in ("/opt/trn_rl_repo",):
    if _p not in sys.path:
        sys.path.insert(0, _p)

import ml_dtypes

BF16 = ml_dtypes.bfloat16

S = 2048
D = 512
P = 128
B = 8
NB = S // P      # 16 key blocks of 128
DC = D // P      # 4 d-chunks of 128
EC = D // P      # 4 e-chunks (contraction for projections)
QW = 512         # tile width (free dim per matmul)
QT = S // QW     # 4 q tiles
SB = QW // P     # 4 s-blocks per q tile
NS = S // QW     # 4 s-slices for the prologue
SCALE = 1.0 / math.sqrt(D)

_compiled = None


def _build():
    import concourse.bass as bass  # noqa: F401
    import concourse.mybir as mybir
    import concourse.tile as tile
    from concourse import bacc

    f32 = mybir.dt.float32
    bf16 = mybir.dt.bfloat16
    Alu = mybir.AluOpType
    Act = mybir.ActivationFunctionType

    nc = bacc.Bacc("TRN2", target_bir_lowering=False, debug=False, num_devices=B)

    # All large inputs arrive packed in their exact per-partition SBUF layout
    # (host does transpose/cast/shuffle): each partition's data is one
    # contiguous run, so DMAs use maximum-size packets on a single queue.
    h1t_d = nc.dram_tensor("h1t", [P, NS, EC, QW], bf16, kind="ExternalInput").ap()
    h2t_d = nc.dram_tensor("h2t", [P, NS, EC, QW], bf16, kind="ExternalInput").ap()
    w_dram = {
        name: nc.dram_tensor(f"{name}_t", [P, EC * D], bf16, kind="ExternalInput").ap()
        for name in ("wq", "wk", "wv", "wo")
    }
    cos_t = nc.dram_tensor("cos_t", [P, 2 * S], bf16, kind="ExternalInput").ap()
    sin_t = nc.dram_tensor("sin_t", [P, 2 * S], bf16, kind="ExternalInput").ap()
    bq_c = nc.dram_tensor("bq_c", [P, DC], f32, kind="ExternalInput").ap()
    bk_c = nc.dram_tensor("bk_c", [P, DC], f32, kind="ExternalInput").ap()
    # bo_b holds bo_eff = bo + Wo @ bv (bv folded through the value path on host)
    bo_b = nc.dram_tensor("bo_b", [P, D], f32, kind="ExternalInput").ap()
    # out[p, qt, sb, d] = y[qt*512 + sb*128 + p, d]: per-partition runs of
    # 2KB+ so the output DMAs move big packets (row-major [S, D] rows were
    # 1KB packets at ~50 GB/s and dominated the kernel tail)
    out = nc.dram_tensor("out", [P, QT, SB, D], bf16, kind="ExternalOutput").ap()

    with tile.TileContext(nc) as tc:
        from contextlib import ExitStack

        with ExitStack() as ctx:
            singles = ctx.enter_context(tc.tile_pool(name="singles", bufs=1))

            # --- persistent tiles ------------------------------------------
            w_sb = {
                name: singles.tile([P, EC, D], bf16, tag=f"w_{name}", name=f"w_{name}")
                for name in ("wq", "wk", "wv", "wo")
            }
            kt_p = [
                singles.tile([P, DC, QW], bf16, tag=f"kt{i}", name=f"kt{i}")
                for i in range(NS)
            ]
            qt_p = [
                singles.tile([P, DC, QW], bf16, tag=f"qt{i}", name=f"qt{i}")
                for i in range(NS)
            ]
            v_p = [
                singles.tile([P, SB, QW], bf16, tag=f"v{i}", name=f"v{i}")
                for i in range(NS)
            ]
            h1s = [
                singles.tile([P, EC, QW], bf16, tag=f"h1s{i}", name=f"h1s{i}")
                for i in range(NS)
            ]
            h2s = [
                singles.tile([P, EC, QW], bf16, tag=f"h2s{i}", name=f"h2s{i}")
                for i in range(NS)
            ]
            cos_sb = singles.tile([P, 2, S], bf16, tag="cos")
            sin_sb = singles.tile([P, 2, S], bf16, tag="sin")
            bq_sb = singles.tile([P, DC], f32, tag="bq")
            bk_sb = singles.tile([P, DC], f32, tag="bk")
            bo_sb = singles.tile([P, D], f32, tag="bo")
            ones_f = singles.tile([P, 1], f32, tag="ones_f")

            # ---- prologue DMAs: three queues (sync/scalar/gpsimd), ordered
            # by first consumption; the critical first-matmul operands (wk
            # halves + first h2 slice halves) lead the earliest-starting
            # sync queue in exact consumption order.
            cos_r = cos_t.rearrange("p (c s) -> p c s", s=S)
            sin_r = sin_t.rearrange("p (c s) -> p c s", s=S)
            wk_r = w_dram["wk"].rearrange("p (c d) -> p c d", d=D)
            # the first-matmul operands issue IN PARALLEL as the head of each
            # queue (each dma_start costs ~0.7us of issue time on its engine,
            # so sequencing them on one queue delays the last by ~3us); the
            # very first ec0 chunks are 128KiB so the first matmul can start
            # after a single chunk lands on sync + scalar
            nc.sync.dma_start(out=h2s[0][:, 0:1, :], in_=h2t_d[:, 0, 0:1, :])
            nc.scalar.dma_start(out=w_sb["wk"][:, 0:1, :], in_=wk_r[:, 0:1, :])
            nc.sync.dma_start(out=h2s[0][:, 1:2, :], in_=h2t_d[:, 0, 1:2, :])
            nc.scalar.dma_start(out=w_sb["wk"][:, 1:2, :], in_=wk_r[:, 1:2, :])
            nc.sync.dma_start(out=h2s[0][:, 2:4, :], in_=h2t_d[:, 0, 2:4, :])
            nc.scalar.dma_start(out=w_sb["wk"][:, 2:4, :], in_=wk_r[:, 2:4, :])
            nc.sync.dma_start(out=h2s[1], in_=h2t_d[:, 1])
            nc.sync.dma_start(out=h2s[3], in_=h2t_d[:, 3])
            # scalar: remaining weights + h2 slice 2 + first h1 slice
            nc.scalar.dma_start(
                out=w_sb["wv"], in_=w_dram["wv"].rearrange("p (c d) -> p c d", d=D)
            )
            nc.scalar.dma_start(
                out=w_sb["wq"], in_=w_dram["wq"].rearrange("p (c d) -> p c d", d=D)
            )
            nc.scalar.dma_start(out=h2s[2], in_=h2t_d[:, 2])
            nc.scalar.dma_start(out=h1s[0], in_=h1t_d[:, 0])
            nc.scalar.dma_start(
                out=w_sb["wo"], in_=w_dram["wo"].rearrange("p (c d) -> p c d", d=D)
            )
            nc.scalar.dma_start(out=bq_sb, in_=bq_c)
            nc.scalar.dma_start(out=bo_sb, in_=bo_b)
            # gpsimd: bias + slice-0 RoPE table chunks (first STT operands),
            # then the table tails
            nc.gpsimd.dma_start(out=bk_sb, in_=bk_c)
            nc.gpsimd.dma_start(out=cos_sb[:, 0, 0:QW], in_=cos_r[:, 0, 0:QW])
            nc.gpsimd.dma_start(out=sin_sb[:, 0, 0:QW], in_=sin_r[:, 0, 0:QW])
            nc.gpsimd.dma_start(out=cos_sb[:, 1, 0:QW], in_=cos_r[:, 1, 0:QW])
            nc.gpsimd.dma_start(out=sin_sb[:, 1, 0:QW], in_=sin_r[:, 1, 0:QW])
            nc.gpsimd.dma_start(out=sin_sb[:, 0, QW:], in_=sin_r[:, 0, QW:])
            nc.gpsimd.dma_start(out=cos_sb[:, 0, QW:], in_=cos_r[:, 0, QW:])
            nc.gpsimd.dma_start(out=sin_sb[:, 1, QW:], in_=sin_r[:, 1, QW:])
            nc.gpsimd.dma_start(out=cos_sb[:, 1, QW:], in_=cos_r[:, 1, QW:])
            nc.vector.memset(ones_f, 1.0)

            # ---------------- Phase A: k/v projections + RoPE ----------------
            ptmp = ctx.enter_context(tc.tile_pool(name="ptmp", bufs=3))

            def project_rope(ht, wname, b_sb, dst, s2, pp_pool, pp_tag):
                # dst[:, dc, :] = RoPE(W @ h^T + b) for columns of slice s2
                sl = slice(s2 * QW, (s2 + 1) * QW)
                for pair in range(2):
                    dc0, dc2 = pair, pair + 2
                    ppa = pp_pool.tile([P, QW], f32, tag=pp_tag, name="ppa")
                    ppb = pp_pool.tile([P, QW], f32, tag=pp_tag, name="ppb")
                    # ec-outer so the first slice consumes the chunked wk/h2
                    # prologue DMAs in arrival order
                    for ec in range(EC):
                        for pp, dc in ((ppa, dc0), (ppb, dc2)):
                            nc.tensor.matmul(
                                pp,
                                lhsT=w_sb[wname][:, ec, dc * P : (dc + 1) * P],
                                rhs=ht[:, ec, :],
                                start=(ec == 0),
                                stop=(ec == EC - 1),
                            )
                    # rope: out[d<256] = x0*cos0 - x2*sin0
                    #       out[d>=256] = x2*cos2 + x0*sin2
                    # (bias folds into the STT's scalar add; the combines
                    # run on the otherwise-idle GpSimd engine)
                    cps = cos_sb[:, pair, sl]
                    sps = sin_sb[:, pair, sl]
                    t0 = ptmp.tile([P, QW], f32, tag="rope0")
                    nc.vector.scalar_tensor_tensor(
                        t0, in0=ppa, scalar=b_sb[:, dc0 : dc0 + 1], in1=cps,
                        op0=Alu.add, op1=Alu.mult,
                    )
                    t1 = ptmp.tile([P, QW], f32, tag="rope1")
                    nc.vector.scalar_tensor_tensor(
                        t1, in0=ppb, scalar=b_sb[:, dc2 : dc2 + 1], in1=sps,
                        op0=Alu.add, op1=Alu.mult,
                    )
                    nc.gpsimd.tensor_tensor(dst[:, dc0, :], t0, t1, Alu.subtract)
                    t2 = ptmp.tile([P, QW], f32, tag="rope0")
                    nc.vector.scalar_tensor_tensor(
                        t2, in0=ppb, scalar=b_sb[:, dc2 : dc2 + 1], in1=cps,
                        op0=Alu.add, op1=Alu.mult,
                    )
                    t3 = ptmp.tile([P, QW], f32, tag="rope1")
                    nc.vector.scalar_tensor_tensor(
                        t3, in0=ppa, scalar=b_sb[:, dc0 : dc0 + 1], in1=sps,
                        op0=Alu.add, op1=Alu.mult,
                    )
                    nc.gpsimd.tensor_tensor(dst[:, dc2, :], t2, t3, Alu.add)

            def project_v(s2, psA):
                # bv is folded into bo on host (bo_eff = bo + Wo @ bv), so
                # this is a plain PSUM->SBUF cast on the idle ACT engine
                for j in range(SB):
                    vp = psA.tile([P, QW], f32, tag="vp")
                    for ec in range(EC):
                        nc.tensor.matmul(
                            vp,
                            lhsT=h2s[s2][:, ec, j * P : (j + 1) * P],
                            rhs=w_sb["wv"][:, ec, :],
                            start=(ec == 0),
                            stop=(ec == EC - 1),
                        )
                    nc.scalar.copy(v_p[s2][:, j, :], vp)

            with tc.tile_pool(
                name="psPP", bufs=6, space="PSUM"
            ) as psPP, tc.tile_pool(name="psVP", bufs=2, space="PSUM") as psVP:
                for s2 in range(NS):
                    project_rope(h2s[s2], "wk", bk_sb, kt_p[s2], s2, psPP, "pp")
                    if s2 == 2:
                        # q slice 0 before the last k slice: its RoPE clears
                        # the DVE before attention needs qt_p[0]
                        project_rope(h1s[0], "wq", bq_sb, qt_p[0], 0, psPP, "pp")
                    project_v(s2, psVP)

            # ---------------- Phase B: attention -----------------------------
            # PSUM budget (8 banks): st 3 + ot 2 + pf 3.
            # PV accumulates in TWO passes of 2 d-chunks each (pt tiles stay
            # resident); the pf rotation serves the NEXT q slice's projection,
            # the denominator columns, and the final projection.
            with tc.tile_pool(name="ptpool", bufs=NB + 2) as ptp, tc.tile_pool(
                name="otsb", bufs=2
            ) as otp, tc.tile_pool(name="outst", bufs=3) as outp, tc.tile_pool(
                name="accp", bufs=2
            ) as accp, tc.tile_pool(
                name="psum_st", bufs=3, space="PSUM"
            ) as ps_st, tc.tile_pool(
                name="psum_ot", bufs=1, space="PSUM"
            ) as ps_ot, tc.tile_pool(name="psum_pf", bufs=3, space="PSUM") as ps_pf:
                for qt in range(QT):
                    if qt + 1 < QT:
                        # project+RoPE the next q slice during this attention;
                        # its h1 slice was DMA'd one iteration earlier
                        nc.sync.dma_start(out=h1s[qt + 1], in_=h1t_d[:, qt + 1])
                        project_rope(
                            h1s[qt + 1], "wq", bq_sb, qt_p[qt + 1], qt + 1, ps_pf, "pf"
                        )

                    ot_sb = otp.tile([P, DC, QW], bf16, tag="ot_sb")
                    accA = accp.tile([P, QW], f32, tag="accA")
                    accB = accp.tile([P, QW], f32, tag="accB")
                    pts = []

                    def emit_pv(pt, ot, kb, dcs):
                        for i, dc in enumerate(dcs):
                            nc.tensor.matmul(
                                ot[:, i, :],
                                lhsT=v_p[kb // SB][:, kb % SB, dc * P : (dc + 1) * P],
                                rhs=pt,
                                start=(kb == 0),
                                stop=(kb == NB - 1),
                            )

                    # pass 1: S^T + exp + colsum-accumulate + PV dc 0,1;
                    # PV(kb-2) is emitted after S^T(kb) so the PE never
                    # head-of-line blocks on exp(kb)
                    ot01 = ps_ot.tile([P, 2, QW], f32, tag="ot", name="ot01")
                    for kb in range(NB):
                        st = ps_st.tile([P, QW], f32, tag="st")
                        for dc in range(DC):
                            nc.tensor.matmul(
                                st,
                                lhsT=kt_p[kb // SB][:, dc, (kb % SB) * P : (kb % SB + 1) * P],
                                rhs=qt_p[qt][:, dc, :],
                                start=(dc == 0),
                                stop=(dc == DC - 1),
                            )
                        pt = ptp.tile([P, QW], bf16, tag="pt")
                        nc.scalar.activation(pt, st, Act.Exp, scale=SCALE)
                        pts.append(pt)
                        # colsum accumulation off the PE: DVE takes even kb,
                        # Pool odd kb; first use per engine initializes (copy)
                        eng, acc = (nc.vector, accA) if kb % 2 == 0 else (nc.gpsimd, accB)
                        if kb < 2:
                            eng.tensor_copy(out=acc, in_=pt)
                        else:
                            eng.tensor_tensor(acc, acc, pt, Alu.add)
                        if kb >= 2:
                            emit_pv(pts[kb - 2], ot01, kb - 2, (0, 1))
                    emit_pv(pts[NB - 2], ot01, NB - 2, (0, 1))
                    emit_pv(pts[NB - 1], ot01, NB - 1, (0, 1))
                    for dc in range(2):
                        nc.any.tensor_copy(out=ot_sb[:, dc, :], in_=ot01[:, dc, :])

                    # pass 2: PV dc 2,3 from the resident pt tiles
                    ot23 = ps_ot.tile([P, 2, QW], f32, tag="ot", name="ot23")
                    for kb in range(NB):
                        emit_pv(pts[kb], ot23, kb, (2, 3))

                    # denominators: 8 one-column fp32 matmuls acc^T @ ones sum
                    # both partial accumulators straight into PSUM with the
                    # sums on q-partitions (no DVE combine / ACT cast in the
                    # chain); reciprocal is a wide [128,4] DVE op. Emitted
                    # AFTER pass 2 so the PE reaches them once the accumulate
                    # chain has long finished.
                    rr = ps_pf.tile([P, SB], f32, tag="pf", name="rr")
                    for sb in range(SB):
                        for acc, last in ((accA, False), (accB, True)):
                            nc.tensor.matmul(
                                rr[:, sb : sb + 1],
                                lhsT=acc[:, sb * P : (sb + 1) * P],
                                rhs=ones_f,
                                start=(not last),
                                stop=last,
                            )
                    r4r = outp.tile([P, SB], f32, tag="r4r")
                    nc.vector.reciprocal(r4r, rr)
                    nc.scalar.copy(ot_sb[:, 2, :], ot23[:, 0, :])
                    nc.vector.tensor_copy(out=ot_sb[:, 3, :], in_=ot23[:, 1, :])

                    # final projection back to natural [s, d] layout; fused
                    # (fp * r) + bo in one DVE op; bf16 store (host casts
                    # back). Staged per-qt so each out DMA moves 2KB/partition
                    # packets in sb pairs.
                    o_sb4 = outp.tile([P, SB, D], bf16, tag="ostage")
                    last_qt = qt == QT - 1
                    for sb in range(SB):
                        # the last q tile's final projections rotate through
                        # the now-idle st banks (4 distinct banks -> no WAR
                        # serialization in the kernel tail)
                        fpool, ftag = (ps_st, "st") if last_qt else (ps_pf, "pf")
                        fp = fpool.tile([P, QW], f32, tag=ftag, name="fp")
                        for dc in range(DC):
                            nc.tensor.matmul(
                                fp,
                                lhsT=ot_sb[:, dc, sb * P : (sb + 1) * P],
                                rhs=w_sb["wo"][:, dc, :],
                                start=(dc == 0),
                                stop=(dc == DC - 1),
                            )
                        nc.vector.scalar_tensor_tensor(
                            o_sb4[:, sb, :],
                            in0=fp,
                            scalar=r4r[:, sb : sb + 1],
                            in1=bo_sb,
                            op0=Alu.mult,
                            op1=Alu.add,
                        )
                        if last_qt:
                            # tail: one DMA per sb, spread across three queues
                            # so the final stores stream in parallel
                            eng = (nc.scalar, nc.sync, nc.gpsimd, nc.scalar)[sb]
                            eng.dma_start(
                                out=out[:, qt, sb : sb + 1, :],
                                in_=o_sb4[:, sb : sb + 1, :],
                            )
                        elif sb % 2 == 1:
                            nc.sync.dma_start(
                                out=out[:, qt, sb - 1 : sb + 1, :],
                                in_=o_sb4[:, sb - 1 : sb + 1, :],
                            )

    nc.compile()
    return nc


def _get_compiled():
    global _compiled
    if _compiled is None:
        _compiled = _build()
    return _compiled


def _pack(x_t, nchunks):
    # [nchunks*P, S] -> [P, nchunks*S]: partition p holds chunks contiguously,
    # matching the SBUF tile layout exactly (max-size DMA packets)
    n = x_t.shape[1]
    return np.ascontiguousarray(
        x_t.reshape(nchunks, P, n).transpose(1, 0, 2).reshape(P, nchunks * n)
    )


def _host_tables():
    half = D // 2
    inv_freq = 1.0 / (10000.0 ** (np.arange(half, dtype=np.float32) / half))
    t = np.arange(S, dtype=np.float32)
    freqs = np.outer(t, inv_freq)
    emb = np.concatenate([freqs, freqs], axis=-1)  # [S, D]
    # the two d-halves of emb are identical - ship only [D/2, S] worth
    cos_t = _pack(np.cos(emb).T[: D // 2].astype(BF16), 2)
    sin_t = _pack(np.sin(emb).T[: D // 2].astype(BF16), 2)
    return cos_t, sin_t


def make_in_maps(**inputs):
    cos_t, sin_t = _host_tables()
    shared = {
        "cos_t": cos_t,
        "sin_t": sin_t,
        "wq_t": _pack(np.asarray(inputs["Wq"], np.float32).T.astype(BF16), EC),
        "wk_t": _pack(np.asarray(inputs["Wk"], np.float32).T.astype(BF16), EC),
        "wv_t": _pack(np.asarray(inputs["Wv"], np.float32).T.astype(BF16), EC),
        "wo_t": _pack(np.asarray(inputs["Wo"], np.float32).T.astype(BF16), EC),
        "bq_c": np.ascontiguousarray(np.asarray(inputs["bq"], np.float32).reshape(DC, P).T),
        "bk_c": np.ascontiguousarray(np.asarray(inputs["bk"], np.float32).reshape(DC, P).T),
        # bv contributes bv @ Wo.T to every output row - fold it into bo
        "bo_b": np.ascontiguousarray(
            np.broadcast_to(
                np.asarray(inputs["bo"], np.float32)
                + np.asarray(inputs["Wo"], np.float32)
                @ np.asarray(inputs["bv"], np.float32),
                (P, D),
            )
        ),
    }
    h1 = np.asarray(inputs["h1"], np.float32)
    h2 = np.asarray(inputs["h2"], np.float32)

    def _pack_h(h):
        # [S, D] -> [P, NS, EC, QW]: t[p, s2, ec, sq] = h[s2*QW+sq, ec*P+p]
        ht = h.T.astype(BF16)  # [D, S]
        return np.ascontiguousarray(
            ht.reshape(EC, P, NS, QW).transpose(1, 2, 0, 3)
        )

    return [
        dict(shared, h1t=_pack_h(h1[core]), h2t=_pack_h(h2[core]))
        for core in range(B)
    ]


def _install_ntff_hook():
    """The agent image's antenv lacks axon_hooks; rebuild the NTFF profile hook
    from libaxon_pjrt.so (mirrors trn_agent_boot._ntff_profile_via_ctypes)."""
    try:
        from antenv.axon_hooks import get_axon_ntff_profile_hook  # noqa: F401

        return
    except ImportError:
        pass
    import contextlib
    import ctypes
    import types

    so_path = "/opt/axon/libaxon_pjrt.so"
    try:
        lib = ctypes.CDLL(so_path)
    except OSError:
        return
    if not hasattr(lib, "axon_start_nrt_profile"):
        return
    lib.axon_start_nrt_profile.argtypes = [
        ctypes.POINTER(ctypes.c_int64),
        ctypes.c_size_t,
    ]
    lib.axon_start_nrt_profile.restype = ctypes.c_int64
    lib.axon_stop_nrt_profile.argtypes = [ctypes.c_char_p]
    lib.axon_stop_nrt_profile.restype = ctypes.c_int64

    @contextlib.contextmanager
    def _hook(output_dir, device_ids):
        import jax

        jax.devices()
        if device_ids:
            ids = (ctypes.c_int64 * len(device_ids))(*device_ids)
            rc = lib.axon_start_nrt_profile(ids, len(device_ids))
        else:
            rc = lib.axon_start_nrt_profile(None, 0)
        if rc != 0:
            raise RuntimeError(f"axon_start_nrt_profile rc={rc}")
        try:
            yield
        finally:
            n = lib.axon_stop_nrt_profile(str(output_dir).encode())
            print(f"ntff profile: {n} file(s) written to {output_dir}")

    import antenv

    mod = types.ModuleType("antenv.axon_hooks")
    mod.get_axon_ntff_profile_hook = lambda: _hook
    mod.set_axon_ntff_profile_hook = lambda h: None
    sys.modules["antenv.axon_hooks"] = mod
    antenv.axon_hooks = mod


def run(trace=False, tmpdir=None, trace_cores=None, **inputs):
    from concourse.bass_utils import run_bass_kernel_spmd

    if trace:
        _install_ntff_hook()
    nc = _get_compiled()
    in_maps = make_in_maps(**inputs)
    kwargs = {}
    if tmpdir is not None:
        kwargs["tmpdir"] = tmpdir
    if trace_cores is not None:
        kwargs["trace_cores"] = trace_cores
    res = run_bass_kernel_spmd(
        nc, in_maps, core_ids=list(range(B)), trace=trace, **kwargs
    )
    # out[p, qt, sb, d] -> y[qt*512 + sb*128 + p, d], cast bf16 -> fp32
    out = np.stack(
        [
            np.asarray(res.results[i]["out"])
            .transpose(1, 2, 0, 3)
            .reshape(S, D)
            for i in range(B)
        ]
    ).astype(np.float32)
    return out, res


def kernel(**inputs):
    out, _ = run(trace=False, **inputs)
    return out


# revision 22
# speedup vs baseline: 1.0431x; 1.0431x over previous
"""Trainium2 Bass kernel for nn_CrossAttention (B=8, S1=S2=2048, D=512, single head).

Sharding: batch dim B=8 across the 8 NeuronCores (data parallel). Each core runs
the full cross-attention for one batch element:
    q = RoPE(h1 @ Wq.T + bq); k = RoPE(h2 @ Wk.T + bk); v = h2 @ Wv.T + bv
    out = softmax(q k^T / sqrt(D)) v @ Wo.T + bo

Design notes (v2):
  - All matmuls in bf16 (fp32 PSUM accumulation): rel_l2 vs fp32 reference ~6e-3.
  - Scores are computed TRANSPOSED (S^T[k,q]) so the probability matrix feeds the
    PV matmul directly as the moving operand - no P transposes.
  - Softmax skips max-subtraction (energies are ~N(0,1), |e| < 8, exp is safe).
  - Softmax denominators WITHOUT PE ones-matmuls: P^T tiles are accumulated
    across key blocks on DVE (even kb) and Pool (odd kb) into two fp32 SBUF
    accumulators; combined + cast to bf16, then 4 one-column matmuls
    (acc_chunk^T @ ones) drop the denominators directly onto q-partitions for
    the reciprocal. This removes 16 512-wide PE matmuls + 4 PE transposes/qt.
  - PSUM (8 banks): st 3 | ot 2 | pf 3. The "pf" tag is a 3-slot rotation of
    1-bank tiles shared by the interleaved next-q projection halves, the
    denominator column tile, and the final-projection accumulators - no
    single-buffer interlock (v1's ps_fin bufs=1 serialized PE against DVE).
  - PV is emitted with a 2-iteration skew behind the score matmuls so the ACT
    exp latency never head-of-line blocks the PE.
  - Prologue DMAs are split across all 5 engine queues with the first-needed
    chunks (wk, first h2 slice halves, first RoPE table slices) leading their
    queues: the first matmul can start as soon as ~2 chunks land instead of
    waiting for whole-tensor transfers.
  - Output is written to DRAM in bf16 (halves the tail DMA) and cast to fp32
    on host.
"""

import math
import sys

import numpy as np

for _p in ("/opt/trn_rl_repo",):
    if _p not in sys.path:
        sys.path.insert(0, _p)

import ml_dtypes

BF16 = ml_dtypes.bfloat16

S = 2048
D = 512
P = 128
B = 8
NB = S // P      # 16 key blocks of 128
DC = D // P      # 4 d-chunks of 128
EC = D // P      # 4 e-chunks (contraction for projections)
QW = 512         # tile width (free dim per matmul)
QT = S // QW     # 4 q tiles
SB = QW // P     # 4 s-blocks per q tile
NS = S // QW     # 4 s-slices for the prologue
SCALE = 1.0 / math.sqrt(D)

_compiled = None


def _build():
    import concourse.bass as bass  # noqa: F401
    import concourse.mybir as mybir
    import concourse.tile as tile
    from concourse import bacc

    f32 = mybir.dt.float32
    bf16 = mybir.dt.bfloat16
    Alu = mybir.AluOpType
    Act = mybir.ActivationFunctionType

    nc = bacc.Bacc("TRN2", target_bir_lowering=False, debug=False, num_devices=B)

    # All large inputs arrive packed in their exact per-partition SBUF layout
    # (host does transpose/cast/shuffle): each partition's data is one
    # contiguous run, so DMAs use maximum-size packets on a single queue.
    h1t_d = nc.dram_tensor("h1t", [P, NS, EC, QW], bf16, kind="ExternalInput").ap()
    h2t_d = nc.dram_tensor("h2t", [P, NS, EC, QW], bf16, kind="ExternalInput").ap()
    w_dram = {
        name: nc.dram_tensor(f"{name}_t", [P, EC * D], bf16, kind="ExternalInput").ap()
        for name in ("wq", "wk", "wv", "wo")
    }
    cos_t = nc.dram_tensor("cos_t", [P, 2 * S], bf16, kind="ExternalInput").ap()
    sin_t = nc.dram_tensor("sin_t", [P, 2 * S], bf16, kind="ExternalInput").ap()
    bq_c = nc.dram_tensor("bq_c", [P, DC], f32, kind="ExternalInput").ap()
    bk_c = nc.dram_tensor("bk_c", [P, DC], f32, kind="ExternalInput").ap()
    # bo_b holds bo_eff = bo + Wo @ bv (bv folded through the value path on host)
    bo_b = nc.dram_tensor("bo_b", [P, D], f32, kind="ExternalInput").ap()
    # out[p, qt, sb, d] = y[qt*512 + sb*128 + p, d]: per-partition runs of
    # 2KB+ so the output DMAs move big packets (row-major [S, D] rows were
    # 1KB packets at ~50 GB/s and dominated the kernel tail)
    out = nc.dram_tensor("out", [P, QT, SB, D], bf16, kind="ExternalOutput").ap()

    with tile.TileContext(nc) as tc:
        from contextlib import ExitStack

        with ExitStack() as ctx:
            singles = ctx.enter_context(tc.tile_pool(name="singles", bufs=1))

            # --- persistent tiles ------------------------------------------
            w_sb = {
                name: singles.tile([P, EC, D], bf16, tag=f"w_{name}", name=f"w_{name}")
                for name in ("wq", "wk", "wv", "wo")
            }
            kt_p = [
                singles.tile([P, DC, QW], bf16, tag=f"kt{i}", name=f"kt{i}")
                for i in range(NS)
            ]
            qt_p = [
                singles.tile([P, DC, QW], bf16, tag=f"qt{i}", name=f"qt{i}")
                for i in range(NS)
            ]
            v_p = [
                singles.tile([P, SB, QW], bf16, tag=f"v{i}", name=f"v{i}")
                for i in range(NS)
            ]
            h1s = [
                singles.tile([P, EC, QW], bf16, tag=f"h1s{i}", name=f"h1s{i}")
                for i in range(NS)
            ]
            h2s = [
                singles.tile([P, EC, QW], bf16, tag=f"h2s{i}", name=f"h2s{i}")
                for i in range(NS)
            ]
            cos_sb = singles.tile([P, 2, S], bf16, tag="cos")
            sin_sb = singles.tile([P, 2, S], bf16, tag="sin")
            bq_sb = singles.tile([P, DC], f32, tag="bq")
            bk_sb = singles.tile([P, DC], f32, tag="bk")
            bo_sb = singles.tile([P, D], f32, tag="bo")
            ones_bf = singles.tile([P, 1], bf16, tag="ones_bf")

            # ---- prologue DMAs: three queues (sync/scalar/gpsimd), ordered
            # by first consumption; the critical first-matmul operands (wk
            # halves + first h2 slice halves) lead the earliest-starting
            # sync queue in exact consumption order.
            cos_r = cos_t.rearrange("p (c s) -> p c s", s=S)
            sin_r = sin_t.rearrange("p (c s) -> p c s", s=S)
            wk_r = w_dram["wk"].rearrange("p (c d) -> p c d", d=D)
            # the first-matmul operands issue IN PARALLEL as the head of each
            # queue (each dma_start costs ~0.7us of issue time on its engine,
            # so sequencing them on one queue delays the last by ~3us); the
            # very first ec0 chunks are 128KiB so the first matmul can start
            # after a single chunk lands on sync + scalar
            nc.sync.dma_start(out=h2s[0][:, 0:1, :], in_=h2t_d[:, 0, 0:1, :])
            nc.scalar.dma_start(out=w_sb["wk"][:, 0:1, :], in_=wk_r[:, 0:1, :])
            nc.sync.dma_start(out=h2s[0][:, 1:2, :], in_=h2t_d[:, 0, 1:2, :])
            nc.scalar.dma_start(out=w_sb["wk"][:, 1:2, :], in_=wk_r[:, 1:2, :])
            nc.sync.dma_start(out=h2s[0][:, 2:4, :], in_=h2t_d[:, 0, 2:4, :])
            nc.scalar.dma_start(out=w_sb["wk"][:, 2:4, :], in_=wk_r[:, 2:4, :])
            nc.sync.dma_start(out=h2s[1], in_=h2t_d[:, 1])
            nc.sync.dma_start(out=h2s[3], in_=h2t_d[:, 3])
            # scalar: remaining weights + h2 slice 2 + first h1 slice
            nc.scalar.dma_start(
                out=w_sb["wv"], in_=w_dram["wv"].rearrange("p (c d) -> p c d", d=D)
            )
            nc.scalar.dma_start(
                out=w_sb["wq"], in_=w_dram["wq"].rearrange("p (c d) -> p c d", d=D)
            )
            nc.scalar.dma_start(out=h2s[2], in_=h2t_d[:, 2])
            nc.scalar.dma_start(out=h1s[0], in_=h1t_d[:, 0])
            nc.scalar.dma_start(
                out=w_sb["wo"], in_=w_dram["wo"].rearrange("p (c d) -> p c d", d=D)
            )
            nc.scalar.dma_start(out=bq_sb, in_=bq_c)
            nc.scalar.dma_start(out=bo_sb, in_=bo_b)
            # gpsimd: bias + slice-0 RoPE table chunks (first STT operands),
            # then the table tails
            nc.gpsimd.dma_start(out=bk_sb, in_=bk_c)
            nc.gpsimd.dma_start(out=cos_sb[:, 0, 0:QW], in_=cos_r[:, 0, 0:QW])
            nc.gpsimd.dma_start(out=sin_sb[:, 0, 0:QW], in_=sin_r[:, 0, 0:QW])
            nc.gpsimd.dma_start(out=cos_sb[:, 1, 0:QW], in_=cos_r[:, 1, 0:QW])
            nc.gpsimd.dma_start(out=sin_sb[:, 1, 0:QW], in_=sin_r[:, 1, 0:QW])
            nc.gpsimd.dma_start(out=sin_sb[:, 0, QW:], in_=sin_r[:, 0, QW:])
            nc.gpsimd.dma_start(out=cos_sb[:, 0, QW:], in_=cos_r[:, 0, QW:])
            nc.gpsimd.dma_start(out=sin_sb[:, 1, QW:], in_=sin_r[:, 1, QW:])
            nc.gpsimd.dma_start(out=cos_sb[:, 1, QW:], in_=cos_r[:, 1, QW:])
            nc.vector.memset(ones_bf, 1.0)

            # ---------------- Phase A: k/v projections + RoPE ----------------
            ptmp = ctx.enter_context(tc.tile_pool(name="ptmp", bufs=3))

            def project_rope_pair(ht, wname, b_sb, dst, s2, pair, pp_pool, pp_tag):
                # dst[:, {pair, pair+2}, :] = RoPE(W @ h^T + b), slice s2
                sl = slice(s2 * QW, (s2 + 1) * QW)
                dc0, dc2 = pair, pair + 2
                ppa = pp_pool.tile([P, QW], f32, tag=pp_tag, name="ppa")
                ppb = pp_pool.tile([P, QW], f32, tag=pp_tag, name="ppb")
                # ec-outer so the first slice consumes the chunked wk/h2
                # prologue DMAs in arrival order
                for ec in range(EC):
                    for pp, dc in ((ppa, dc0), (ppb, dc2)):
                        nc.tensor.matmul(
                            pp,
                            lhsT=w_sb[wname][:, ec, dc * P : (dc + 1) * P],
                            rhs=ht[:, ec, :],
                            start=(ec == 0),
                            stop=(ec == EC - 1),
                        )
                # rope: out[d<256] = x0*cos0 - x2*sin0
                #       out[d>=256] = x2*cos2 + x0*sin2
                # (bias folds into the STT's scalar add; the combines
                # run on the otherwise-idle GpSimd engine)
                cps = cos_sb[:, pair, sl]
                sps = sin_sb[:, pair, sl]
                t0 = ptmp.tile([P, QW], f32, tag="rope0")
                nc.vector.scalar_tensor_tensor(
                    t0, in0=ppa, scalar=b_sb[:, dc0 : dc0 + 1], in1=cps,
                    op0=Alu.add, op1=Alu.mult,
                )
                t1 = ptmp.tile([P, QW], f32, tag="rope1")
                nc.vector.scalar_tensor_tensor(
                    t1, in0=ppb, scalar=b_sb[:, dc2 : dc2 + 1], in1=sps,
                    op0=Alu.add, op1=Alu.mult,
                )
                nc.gpsimd.tensor_tensor(dst[:, dc0, :], t0, t1, Alu.subtract)
                t2 = ptmp.tile([P, QW], f32, tag="rope0")
                nc.vector.scalar_tensor_tensor(
                    t2, in0=ppb, scalar=b_sb[:, dc2 : dc2 + 1], in1=cps,
                    op0=Alu.add, op1=Alu.mult,
                )
                t3 = ptmp.tile([P, QW], f32, tag="rope1")
                nc.vector.scalar_tensor_tensor(
                    t3, in0=ppa, scalar=b_sb[:, dc0 : dc0 + 1], in1=sps,
                    op0=Alu.add, op1=Alu.mult,
                )
                nc.gpsimd.tensor_tensor(dst[:, dc2, :], t2, t3, Alu.add)

            def project_rope(ht, wname, b_sb, dst, s2, pp_pool, pp_tag):
                for pair in range(2):
                    project_rope_pair(ht, wname, b_sb, dst, s2, pair, pp_pool, pp_tag)

            def project_v_chunk(s2, j, psV):
                # bv is folded into bo on host (bo_eff = bo + Wo @ bv), so
                # this is a plain PSUM->SBUF cast on the idle ACT engine
                vp = psV.tile([P, QW], f32, tag="vp")
                for ec in range(EC):
                    nc.tensor.matmul(
                        vp,
                        lhsT=h2s[s2][:, ec, j * P : (j + 1) * P],
                        rhs=w_sb["wv"][:, ec, :],
                        start=(ec == 0),
                        stop=(ec == EC - 1),
                    )
                nc.scalar.copy(v_p[s2][:, j, :], vp)

            # Phase A schedule: RoPE pairs (whose 4 DVE STTs take ~3us each)
            # are interleaved with DVE-free v-projection chunks so the PE's
            # pair emission rate (~1.7us of matmuls) never outruns the DVE:
            # un-interleaved, the pp PSUM rotation throttles the PE to DVE
            # pace for the whole k/q-projection stretch.
            with tc.tile_pool(
                name="psPP", bufs=6, space="PSUM"
            ) as psPP, tc.tile_pool(name="psVP", bufs=2, space="PSUM") as psVP:
                K_ = lambda s2, pair: project_rope_pair(
                    h2s[s2], "wk", bk_sb, kt_p[s2], s2, pair, psPP, "pp"
                )
                Q_ = lambda pair: project_rope_pair(
                    h1s[0], "wq", bq_sb, qt_p[0], 0, pair, psPP, "pp"
                )
                V_ = lambda s2, j: project_v_chunk(s2, j, psVP)
                K_(0, 0); K_(0, 1); V_(0, 0)
                K_(1, 0); V_(0, 1); K_(1, 1); V_(0, 2)
                K_(2, 0); V_(0, 3); K_(2, 1); V_(1, 0)
                Q_(0); V_(1, 1); Q_(1); V_(1, 2)
                K_(3, 0); V_(1, 3); K_(3, 1)
                for j in range(SB):
                    V_(2, j)
                for j in range(SB):
                    V_(3, j)

            # ---------------- Phase B: attention -----------------------------
            # PSUM budget (8 banks): st 3 + ot 2 + pf 3.
            # PV accumulates in TWO passes of 2 d-chunks each (pt tiles stay
            # resident); the pf rotation serves the NEXT q slice's projection,
            # the denominator columns, and the final projection.
            with tc.tile_pool(name="ptpool", bufs=NB + 2) as ptp, tc.tile_pool(
                name="otsb", bufs=2
            ) as otp, tc.tile_pool(name="outst", bufs=3) as outp, tc.tile_pool(
                name="accp", bufs=2
            ) as accp, tc.tile_pool(
                name="psum_st", bufs=3, space="PSUM"
            ) as ps_st, tc.tile_pool(
                name="psum_ot", bufs=1, space="PSUM"
            ) as ps_ot, tc.tile_pool(name="psum_pf", bufs=3, space="PSUM") as ps_pf:
                for qt in range(QT):
                    if qt + 1 < QT:
                        # project+RoPE the next q slice during this attention;
                        # its h1 slice was DMA'd one iteration earlier
                        nc.sync.dma_start(out=h1s[qt + 1], in_=h1t_d[:, qt + 1])
                        project_rope(
                            h1s[qt + 1], "wq", bq_sb, qt_p[qt + 1], qt + 1, ps_pf, "pf"
                        )

                    ot_sb = otp.tile([P, DC, QW], bf16, tag="ot_sb")
                    accA = accp.tile([P, QW], f32, tag="accA")
                    accB = accp.tile([P, QW], f32, tag="accB")
                    accAb = accp.tile([P, QW], bf16, tag="accAb")
                    accBb = accp.tile([P, QW], bf16, tag="accBb")
                    pts = []

                    def emit_pv(pt, ot, kb, dcs):
                        for i, dc in enumerate(dcs):
                            nc.tensor.matmul(
                                ot[:, i, :],
                                lhsT=v_p[kb // SB][:, kb % SB, dc * P : (dc + 1) * P],
                                rhs=pt,
                                start=(kb == 0),
                                stop=(kb == NB - 1),
                            )

                    # pass 1: S^T + exp + colsum-accumulate + PV dc 0,1;
                    # PV(kb-2) is emitted after S^T(kb) so the PE never
                    # head-of-line blocks on exp(kb)
                    ot01 = ps_ot.tile([P, 2, QW], f32, tag="ot", name="ot01")
                    for kb in range(NB):
                        st = ps_st.tile([P, QW], f32, tag="st")
                        for dc in range(DC):
                            nc.tensor.matmul(
                                st,
                                lhsT=kt_p[kb // SB][:, dc, (kb % SB) * P : (kb % SB + 1) * P],
                                rhs=qt_p[qt][:, dc, :],
                                start=(dc == 0),
                                stop=(dc == DC - 1),
                            )
                        pt = ptp.tile([P, QW], bf16, tag="pt")
                        nc.scalar.activation(pt, st, Act.Exp, scale=SCALE)
                        pts.append(pt)
                        # colsum accumulation off the PE: DVE takes even kb,
                        # Pool odd kb; first use per engine initializes
                        # (copy); the LAST add of each chain writes bf16 so
                        # the denominator matmuls read it directly (no
                        # combine/cast latency in the PE's path)
                        eng, acc = (nc.vector, accA) if kb % 2 == 0 else (nc.gpsimd, accB)
                        if kb < 2:
                            eng.tensor_copy(out=acc, in_=pt)
                        elif kb >= NB - 2:
                            accb = accAb if kb % 2 == 0 else accBb
                            eng.tensor_tensor(accb, acc, pt, Alu.add)
                        else:
                            eng.tensor_tensor(acc, acc, pt, Alu.add)
                        if kb >= 2:
                            emit_pv(pts[kb - 2], ot01, kb - 2, (0, 1))
                    emit_pv(pts[NB - 2], ot01, NB - 2, (0, 1))
                    emit_pv(pts[NB - 1], ot01, NB - 1, (0, 1))
                    for dc in range(2):
                        nc.any.tensor_copy(out=ot_sb[:, dc, :], in_=ot01[:, dc, :])

                    # pass 2: PV dc 2,3 from the resident pt tiles
                    ot23 = ps_ot.tile([P, 2, QW], f32, tag="ot", name="ot23")
                    for kb in range(NB):
                        emit_pv(pts[kb], ot23, kb, (2, 3))

                    # denominators: 8 one-column bf16 matmuls acc^T @ ones sum
                    # both partial accumulators straight into PSUM with the
                    # sums on q-partitions; reciprocal is a wide [128,4] DVE
                    # op. Emitted AFTER pass 2 so the PE reaches them once
                    # the accumulate chain has long finished.
                    rr = ps_pf.tile([P, SB], f32, tag="pf", name="rr")
                    for sb in range(SB):
                        for acc, last in ((accAb, False), (accBb, True)):
                            nc.tensor.matmul(
                                rr[:, sb : sb + 1],
                                lhsT=acc[:, sb * P : (sb + 1) * P],
                                rhs=ones_bf,
                                start=(not last),
                                stop=last,
                            )
                    r4r = outp.tile([P, SB], f32, tag="r4r")
                    nc.vector.reciprocal(r4r, rr)
                    nc.scalar.copy(ot_sb[:, 2, :], ot23[:, 0, :])
                    nc.vector.tensor_copy(out=ot_sb[:, 3, :], in_=ot23[:, 1, :])

                    # final projection back to natural [s, d] layout; fused
                    # (fp * r) + bo in one DVE op; bf16 store (host casts
                    # back). Staged per-qt so each out DMA moves 2KB/partition
                    # packets in sb pairs.
                    o_sb4 = outp.tile([P, SB, D], bf16, tag="ostage")
                    last_qt = qt == QT - 1
                    for sb in range(SB):
                        # the last q tile's final projections rotate through
                        # the now-idle st banks (4 distinct banks -> no WAR
                        # serialization in the kernel tail)
                        fpool, ftag = (ps_st, "st") if last_qt else (ps_pf, "pf")
                        fp = fpool.tile([P, QW], f32, tag=ftag, name="fp")
                        for dc in range(DC):
                            nc.tensor.matmul(
                                fp,
                                lhsT=ot_sb[:, dc, sb * P : (sb + 1) * P],
                                rhs=w_sb["wo"][:, dc, :],
                                start=(dc == 0),
                                stop=(dc == DC - 1),
                            )
                        nc.vector.scalar_tensor_tensor(
                            o_sb4[:, sb, :],
                            in0=fp,
                            scalar=r4r[:, sb : sb + 1],
                            in1=bo_sb,
                            op0=Alu.mult,
                            op1=Alu.add,
                        )
                        if last_qt:
                            # tail: one DMA per sb, spread across three queues
                            # so the final stores stream in parallel
                            eng = (nc.scalar, nc.sync, nc.gpsimd, nc.scalar)[sb]
                            eng.dma_start(
                                out=out[:, qt, sb : sb + 1, :],
                                in_=o_sb4[:, sb : sb + 1, :],
                            )
                        elif sb % 2 == 1:
                            # rotate queues so all three stay warm for the tail
                            eng = (
                                (nc.scalar, nc.sync),
                                (nc.gpsimd, nc.scalar),
                                (nc.sync, nc.gpsimd),
                            )[qt][sb // 2]
                            eng.dma_start(
                                out=out[:, qt, sb - 1 : sb + 1, :],
                                in_=o_sb4[:, sb - 1 : sb + 1, :],
                            )

    nc.compile()
    return nc


def _get_compiled():
    global _compiled
    if _compiled is None:
        _compiled = _build()
    return _compiled


def _pack(x_t, nchunks):
    # [nchunks*P, S] -> [P, nchunks*S]: partition p holds chunks contiguously,
    # matching the SBUF tile layout exactly (max-size DMA packets)
    n = x_t.shape[1]
    return np.ascontiguousarray(
        x_t.reshape(nchunks, P, n).transpose(1, 0, 2).reshape(P, nchunks * n)
    )


def _host_tables():
    half = D // 2
    inv_freq = 1.0 / (10000.0 ** (np.arange(half, dtype=np.float32) / half))
    t = np.arange(S, dtype=np.float32)
    freqs = np.outer(t, inv_freq)
    emb = np.concatenate([freqs, freqs], axis=-1)  # [S, D]
    # the two d-halves of emb are identical - ship only [D/2, S] worth
    cos_t = _pack(np.cos(emb).T[: D // 2].astype(BF16), 2)
    sin_t = _pack(np.sin(emb).T[: D // 2].astype(BF16), 2)
    return cos_t, sin_t


def make_in_maps(**inputs):
    cos_t, sin_t = _host_tables()
    shared = {
        "cos_t": cos_t,
        "sin_t": sin_t,
        "wq_t": _pack(np.asarray(inputs["Wq"], np.float32).T.astype(BF16), EC),
        "wk_t": _pack(np.asarray(inputs["Wk"], np.float32).T.astype(BF16), EC),
        "wv_t": _pack(np.asarray(inputs["Wv"], np.float32).T.astype(BF16), EC),
        "wo_t": _pack(np.asarray(inputs["Wo"], np.float32).T.astype(BF16), EC),
        "bq_c": np.ascontiguousarray(np.asarray(inputs["bq"], np.float32).reshape(DC, P).T),
        "bk_c": np.ascontiguousarray(np.asarray(inputs["bk"], np.float32).reshape(DC, P).T),
        # bv contributes bv @ Wo.T to every output row - fold it into bo
        "bo_b": np.ascontiguousarray(
            np.broadcast_to(
                np.asarray(inputs["bo"], np.float32)
                + np.asarray(inputs["Wo"], np.float32)
                @ np.asarray(inputs["bv"], np.float32),
                (P, D),
            )
        ),
    }
    h1 = np.asarray(inputs["h1"], np.float32)
    h2 = np.asarray(inputs["h2"], np.float32)

    def _pack_h(h):
        # [S, D] -> [P, NS, EC, QW]: t[p, s2, ec, sq] = h[s2*QW+sq, ec*P+p]
        ht = h.T.astype(BF16)  # [D, S]
        return np.ascontiguousarray(
            ht.reshape(EC, P, NS, QW).transpose(1, 2, 0, 3)
        )

    return [
        dict(shared, h1t=_pack_h(h1[core]), h2t=_pack_h(h2[core]))
        for core in range(B)
    ]


def _install_ntff_hook():
    """The agent image's antenv lacks axon_hooks; rebuild the NTFF profile hook
    from libaxon_pjrt.so (mirrors trn_agent_boot._ntff_profile_via_ctypes)."""
    try:
        from antenv.axon_hooks import get_axon_ntff_profile_hook  # noqa: F401

        return
    except ImportError:
        pass
    import contextlib
    import ctypes
    import types

    so_path = "/opt/axon/libaxon_pjrt.so"
    try:
        lib = ctypes.CDLL(so_path)
    except OSError:
        return
    if not hasattr(lib, "axon_start_nrt_profile"):
        return
    lib.axon_start_nrt_profile.argtypes = [
        ctypes.POINTER(ctypes.c_int64),
        ctypes.c_size_t,
    ]
    lib.axon_start_nrt_profile.restype = ctypes.c_int64
    lib.axon_stop_nrt_profile.argtypes = [ctypes.c_char_p]
    lib.axon_stop_nrt_profile.restype = ctypes.c_int64

    @contextlib.contextmanager
    def _hook(output_dir, device_ids):
        import jax

        jax.devices()
        if device_ids:
            ids = (ctypes.c_int64 * len(device_ids))(*device_ids)
            rc = lib.axon_start_nrt_profile(ids, len(device_ids))
        else:
            rc = lib.axon_start_nrt_profile(None, 0)
        if rc != 0:
            raise RuntimeError(f"axon_start_nrt_profile rc={rc}")
        try:
            yield
        finally:
            n = lib.axon_stop_nrt_profile(str(output_dir).encode())
            print(f"ntff profile: {n} file(s) written to {output_dir}")

    import antenv

    mod = types.ModuleType("antenv.axon_hooks")
    mod.get_axon_ntff_profile_hook = lambda: _hook
    mod.set_axon_ntff_profile_hook = lambda h: None
    sys.modules["antenv.axon_hooks"] = mod
    antenv.axon_hooks = mod


def run(trace=False, tmpdir=None, trace_cores=None, **inputs):
    from concourse.bass_utils import run_bass_kernel_spmd

    if trace:
        _install_ntff_hook()
    nc = _get_compiled()
    in_maps = make_in_maps(**inputs)
    kwargs = {}
    if tmpdir is not None:
        kwargs["tmpdir"] = tmpdir
    if trace_cores is not None:
        kwargs["trace_cores"] = trace_cores
    res = run_bass_kernel_spmd(
        nc, in_maps, core_ids=list(range(B)), trace=trace, **kwargs
    )
    # out[p, qt, sb, d] -> y[qt*512 + sb*128 + p, d], cast bf16 -> fp32
    out = np.stack(
        [
            np.asarray(res.results[i]["out"])
            .transpose(1, 2, 0, 3)
            .reshape(S, D)
            for i in range(B)
        ]
    ).astype(np.float32)
    return out, res


def kernel(**inputs):
    out, _ = run(trace=False, **inputs)
    return out
